# revision 23
# baseline (speedup 1.0000x reference)
"""Bahdanau additive attention + LayerNorm, distributed over 8 TRN2 NeuronCores.

Data parallel over the batch dim: each core handles 128 of the 1024 batch rows.
Per core (B=128, S=1024, D=H=128):
  - h_s is streamed from HBM once, cast f32->bf16 in-flight by the SWDGE DMA
    (natural [s, d] layout), and a transposed [d, s] copy is produced by the
    DMA xbar transpose so both the U_a projection (contracts d) and the
    context reduction (contracts s) can run on TensorE without any engine
    transpose pass.
  - scores are computed per 128-wide s-chunk with the tanh energy tile as the
    matmul stationary operand, so they land across partitions; softmax runs
    unnormalized (exp then a deferred divide in the epilogue).
"""

import os
import numpy as np
from contextlib import ExitStack

import concourse.bass as bass
import concourse.mybir as mybir
from concourse.bass_utils import run_bass_kernel_spmd
from concourse.tile import TileContext
from concourse.vector_clock import ScopedClock, VectorClock
from concourse.masks import make_identity

# ---------------------------------------------------------------------------
# Workaround for walrus "Too many sync wait commands" on the TileContext final
# Drain: put the end-of-kernel semaphore waits on individual nops (engine
# instructions execute in order, so a bare drain afterwards is equivalent).
# ---------------------------------------------------------------------------


def _patched_drain_and_barrier(self, tick_clock, wait_clock):
    gc = tick_clock.global_clock
    for i, t in enumerate(list(gc)):
        if t > 0:
            pc = VectorClock()
            for _ in range(t):
                pc.advance(i)
            nop_i = self.nc.sync.nop(hint=f"drainwait{i}", nofuse=True)
            wait_clock.add_sem_waits(nop_i.ins, ScopedClock({None: pc}))
    self.nc.sync.drain()
    self.nc.all_engine_barrier()
    assert self.sems is not None
    popped = self.nc._tile_sem_poison_stack.pop()
    assert popped is self._sem_poison
    self.nc.clear_and_free_semaphores(list(self.sems.allocated().values()))
    self.nc.all_engine_barrier()


TileContext._drain_and_barrier = _patched_drain_and_barrier

# ---------------------------------------------------------------------------

NCORES = 8
B = 128          # batch rows per core (1024 / 8)
S = 1024
D = 128
H = 128
EPS = 1e-3

F32 = mybir.dt.float32
BF16 = mybir.dt.bfloat16

GRP = 16         # batch rows per softmax group (one psum scores bank)
NCH = S // 128   # 8 s-chunks

Tanh = mybir.ActivationFunctionType.Tanh
Exp = mybir.ActivationFunctionType.Exp
Sqrt = mybir.ActivationFunctionType.Sqrt
ADD = mybir.AluOpType.add
SUB = mybir.AluOpType.subtract
MULT = mybir.AluOpType.mult
DIV = mybir.AluOpType.divide
AX_X = mybir.AxisListType.X


def _build(debug=False):
    nc = bass.Bass()
    h_t_e = nc.declare_dram_parameter("h_t", [B, D], F32, isOutput=False)
    h_s_e = nc.declare_dram_parameter("h_s", [B, S, D], F32, isOutput=False)
    W_a_e = nc.declare_dram_parameter("W_a", [D, H], F32, isOutput=False)
    U_a_e = nc.declare_dram_parameter("U_a", [D, H], F32, isOutput=False)
    V_a_e = nc.declare_dram_parameter("V_a", [H, 1], F32, isOutput=False)
    W_c_e = nc.declare_dram_parameter("W_c", [2 * D, H], F32, isOutput=False)
    b_c_e = nc.declare_dram_parameter("b_c", [H], F32, isOutput=False)
    gamma_e = nc.declare_dram_parameter("gamma", [H], F32, isOutput=False)
    beta_e = nc.declare_dram_parameter("beta", [H], F32, isOutput=False)
    out_e = nc.declare_dram_parameter("out", [B, H], F32, isOutput=True)
    if debug:
        dbg_hsT = nc.declare_dram_parameter("dbg_hsT", [D, S], F32, isOutput=True)
        dbg_tanh = nc.declare_dram_parameter("dbg_tanh", [H, S], F32, isOutput=True)
        dbg_exp = nc.declare_dram_parameter("dbg_exp", [128, 128], F32, isOutput=True)
        dbg_sums = nc.declare_dram_parameter("dbg_sums", [1, B], F32, isOutput=True)
        dbg_ctx = nc.declare_dram_parameter("dbg_ctx", [B, D], F32, isOutput=True)
        dbg_attn = nc.declare_dram_parameter("dbg_attn", [B, H], F32, isOutput=True)

    with TileContext(nc) as tc, ExitStack() as ctx:
        consts = ctx.enter_context(tc.tile_pool(name="consts", bufs=1))
        nat_pool = ctx.enter_context(tc.tile_pool(name="nat", bufs=24))
        hsT_pool = ctx.enter_context(tc.tile_pool(name="hsT", bufs=3))
        tanh_pool = ctx.enter_context(tc.tile_pool(name="tanh", bufs=3))
        exp_pool = ctx.enter_context(tc.tile_pool(name="expT", bufs=2))
        small = ctx.enter_context(tc.tile_pool(name="small", bufs=4))
        pe_psum = ctx.enter_context(tc.tile_pool(name="pe_psum", bufs=2, space="PSUM"))
        sc_psum = ctx.enter_context(tc.tile_pool(name="sc_psum", bufs=2, space="PSUM"))
        tr_psum = ctx.enter_context(tc.tile_pool(name="tr_psum", bufs=2, space="PSUM"))

        # ----- constants / preamble -----
        identity = consts.tile([128, 128], F32, tag="identity")
        make_identity(nc, identity)
        ident_bf = consts.tile([128, 128], BF16, tag="ident_bf")
        make_identity(nc, ident_bf)
        ones_col = consts.tile([128, 1], F32, tag="ones_col")
        nc.vector.memset(ones_col, 1.0)
        ones_row = consts.tile([1, 128], F32, tag="ones_row")
        nc.vector.memset(ones_row, 1.0)
        eps_col = consts.tile([128, 1], F32, tag="eps_col")
        nc.vector.memset(eps_col, EPS)

        U_bf = consts.tile([D, H], BF16, tag="U_bf")
        nc.gpsimd.dma_start(out=U_bf[:, :], in_=U_a_e[:, :])
        V_bf = consts.tile([H, 1], BF16, tag="V_bf")
        nc.gpsimd.dma_start(out=V_bf[:, :], in_=V_a_e[:, :])
        W_a_s = consts.tile([D, H], F32, tag="W_a_s")
        nc.sync.dma_start(out=W_a_s[:, :], in_=W_a_e[:, :])
        Wc_top = consts.tile([D, H], F32, tag="Wc_top")
        nc.sync.dma_start(out=Wc_top[:, :], in_=W_c_e[0:D, :])
        Wc_bot = consts.tile([D, H], F32, tag="Wc_bot")
        nc.sync.dma_start(out=Wc_bot[:, :], in_=W_c_e[D : 2 * D, :])
        b_c_col = consts.tile([H, 1], F32, tag="b_c_col")
        nc.sync.dma_start(out=b_c_col[:, :], in_=b_c_e[:])
        gamma_row = consts.tile([1, H], F32, tag="gamma_row")
        nc.sync.dma_start(out=gamma_row[:, :], in_=gamma_e[:])
        beta_row = consts.tile([1, H], F32, tag="beta_row")
        nc.sync.dma_start(out=beta_row[:, :], in_=beta_e[:])
        h_t_s = consts.tile([B, D], F32, tag="h_t_s")
        nc.sync.dma_start(out=h_t_s[:, :], in_=h_t_e[:, :])

        # broadcast gamma/beta across partitions: ones[128,1] (x) row[1,128]
        ps_g = sc_psum.tile([128, H], F32, tag="sc")
        nc.tensor.matmul(ps_g[:, :], lhsT=ones_row[:, :], rhs=gamma_row[:, :],
                         start=True, stop=True)
        gamma_b = consts.tile([128, H], F32, tag="gamma_b")
        nc.vector.tensor_copy(out=gamma_b[:, :], in_=ps_g[:, :])
        ps_b = sc_psum.tile([128, H], F32, tag="sc")
        nc.tensor.matmul(ps_b[:, :], lhsT=ones_row[:, :], rhs=beta_row[:, :],
                         start=True, stop=True)
        beta_b = consts.tile([128, H], F32, tag="beta_b")
        nc.vector.tensor_copy(out=beta_b[:, :], in_=ps_b[:, :])

        # htT[d, b] = h_t^T ; ht_projT[h, b] = W_a^T @ htT
        ps_t = sc_psum.tile([D, B], F32, tag="sc")
        nc.tensor.matmul(ps_t[:, :], lhsT=h_t_s[:, :], rhs=identity[:, :],
                         start=True, stop=True)
        htT = consts.tile([D, B], F32, tag="htT")
        nc.vector.tensor_copy(out=htT[:, :], in_=ps_t[:, :])
        ps_p = sc_psum.tile([H, B], F32, tag="sc")
        nc.tensor.matmul(ps_p[:, :], lhsT=W_a_s[:, :], rhs=htT[:, :],
                         start=True, stop=True)
        ht_projT = consts.tile([H, B], F32, tag="ht_projT")
        nc.vector.tensor_copy(out=ht_projT[:, :], in_=ps_p[:, :])

        # persistent accumulators
        ctxT_raw = consts.tile([D, B], F32, tag="ctxT_raw")
        sums_row = consts.tile([1, B], F32, tag="sums_row")

        # ----- main loop over softmax groups of GRP batch rows -----
        # Group psum bank layout (one [128, 512] tile per group, ALL matmuls
        # single-shot so the per-bank has_written bit clearing is harmless):
        #   cols [0, 128)    scoresT: col bl*8+c = scores of row bl, s-chunk c
        #   cols [128, 144)  per-row exp sums (row 0 only)
        #   cols [144, 272)  ctx partials: col 144+bl*8+c = partial ctx^T
        n_groups = B // GRP
        for g in range(n_groups):
            ps_grp = sc_psum.tile([128, 512], F32, tag="sc")
            expT_g = exp_pool.tile([128, GRP * 8], BF16, tag="expT")
            nat_tiles = []
            for bl in range(GRP):
                b = g * GRP + bl
                # 1. HBM load with f32->bf16 cast. Partition p holds the 8
                # consecutive rows s = 8p..8p+7 (4KB contiguous source per
                # partition -> full-size DMA descriptors); "chunk" c is the
                # strided subset {s : s % 8 == c}. Softmax and the context
                # sum are s-order-agnostic, and scores/exp/ctx all use the
                # same chunk labeling, so the permutation is invisible.
                nat_b = nat_pool.tile([128, NCH, D], BF16, tag="nat")
                nc.gpsimd.dma_start(
                    out=nat_b[:, :, :],
                    in_=h_s_e[b].rearrange("(p r) d -> p r d", r=NCH),
                )
                nat_tiles.append(nat_b)
                # 2. transpose via TensorE identity matmuls -> hsT [d, s],
                #    evacuating psum->SBUF(bf16) on DVE (3/4) and ACT (1/4)
                hsT = hsT_pool.tile([D, S], BF16, tag="hsT")
                for half in range(2):
                    ps_tr = tr_psum.tile([D, 512], F32, tag="tr")
                    for cc in range(4):
                        c = half * 4 + cc
                        nc.tensor.matmul(
                            ps_tr[:, cc * 128 : (cc + 1) * 128],
                            lhsT=nat_b[:, c, :], rhs=ident_bf[:, :],
                            start=True, stop=True,
                        )
                    dst = hsT[:, half * 512 : (half + 1) * 512]
                    if (2 * bl + half) % 4 == 0:
                        nc.scalar.copy(out=dst, in_=ps_tr[:, :])
                    else:
                        nc.vector.tensor_copy(out=dst, in_=ps_tr[:, :])
                # 3. energy pre-activation: U_a^T @ h_s^T -> [h, s]
                ps_e = pe_psum.tile([H, S], F32, tag="pe")
                nc.tensor.matmul(ps_e[:, 0:512], lhsT=U_bf[:, :],
                                 rhs=hsT[:, 0:512], start=True, stop=True)
                nc.tensor.matmul(ps_e[:, 512:1024], lhsT=U_bf[:, :],
                                 rhs=hsT[:, 512:1024], start=True, stop=True)
                # 4. tanh(e + ht_proj[b]) with per-partition bias
                tanh_e = tanh_pool.tile([H, S], BF16, tag="tanh")
                nc.scalar.activation(tanh_e[:, :], ps_e[:, :], Tanh,
                                     bias=ht_projT[:, b : b + 1], scale=1.0)
                if debug and b == 0:
                    nc.gpsimd.dma_start(out=dbg_hsT[:, :], in_=hsT[:, :])
                    nc.gpsimd.dma_start(out=dbg_tanh[:, :], in_=tanh_e[:, :])
                # 5. scores: V_a^T tanh_e per chunk; stationary = tanh chunk
                for c in range(NCH):
                    k = bl * 8 + c
                    nc.tensor.matmul(
                        ps_grp[:, k : k + 1],
                        lhsT=tanh_e[:, c * 128 : (c + 1) * 128],
                        rhs=V_bf[:, :],
                        start=True, stop=True,
                    )
            # group tail: exp, per-row sums, context partials
            nc.scalar.activation(expT_g[:, :], ps_grp[:, 0 : GRP * 8], Exp)
            sumP = small.tile([128, GRP], F32, tag="sumP")
            nc.vector.tensor_reduce(
                sumP[:, :],
                expT_g.rearrange("p (b c) -> p b c", c=8),
                axis=AX_X, op=ADD,
            )
            nc.tensor.matmul(ps_grp[0:1, 128 : 128 + GRP], lhsT=ones_col[:, :],
                             rhs=sumP[:, :], start=True, stop=True)
            sums_cp = nc.vector.tensor_copy(
                out=sums_row[:, g * GRP : (g + 1) * GRP],
                in_=ps_grp[0:1, 128 : 128 + GRP])
            if debug and g == 0:
                nc.gpsimd.dma_start(out=dbg_exp[:, :], in_=expT_g[:, :])
            # ctx^T partials: one column per (row, chunk), no accumulation.
            # TensorE writes here must not overlap the DVE read of the sums
            # region in the same psum bank (PSUM collisions are fatal).
            first_ctx = True
            for bl in range(GRP):
                nat_b = nat_tiles[bl]
                for c in range(NCH):
                    k = 144 + bl * 8 + c
                    mm = nc.tensor.matmul(
                        ps_grp[:, k : k + 1],
                        lhsT=nat_b[:, c, :],
                        rhs=expT_g[:, bl * 8 + c : bl * 8 + c + 1],
                        start=True, stop=True,
                    )
                    if first_ctx:
                        bass._add_dep_helper(
                            mm.ins, sums_cp.ins, sync=True,
                            reason="ctx psum writes wait for sums bank read")
                        first_ctx = False
            # reduce the 8 partials per row -> ctx^T[:, g*16:(g+1)*16]
            nc.vector.tensor_reduce(
                ctxT_raw[:, g * GRP : (g + 1) * GRP],
                ps_grp[:, 144 : 144 + GRP * 8].rearrange("p (b c) -> p b c", c=8),
                axis=AX_X, op=ADD,
            )

        # ----- epilogue -----
        # normalize ctx^T by the per-row softmax sums (broadcast along d)
        inv_row = small.tile([1, B], F32, tag="inv_row")
        nc.vector.reciprocal(out=inv_row[:, :], in_=sums_row[:, :])
        ps_ib = sc_psum.tile([128, B], F32, tag="sc")
        nc.tensor.matmul(ps_ib[:, :], lhsT=ones_row[:, :], rhs=inv_row[:, :],
                         start=True, stop=True)
        ctxT = small.tile([D, B], F32, tag="ctxT")
        nc.vector.tensor_tensor(out=ctxT[:, :], in0=ctxT_raw[:, :],
                                in1=ps_ib[:, :], op=MULT)
        # attnT[h, b] = tanh(Wc_top^T ctxT + Wc_bot^T htT + b_c)
        ps_at = sc_psum.tile([H, B], F32, tag="sc")
        nc.tensor.matmul(ps_at[:, :], lhsT=Wc_top[:, :], rhs=ctxT[:, :],
                         start=True, stop=False)
        nc.tensor.matmul(ps_at[:, :], lhsT=Wc_bot[:, :], rhs=htT[:, :],
                         start=False, stop=True)
        attnT = small.tile([H, B], F32, tag="attnT")
        nc.scalar.activation(attnT[:, :], ps_at[:, :], Tanh,
                             bias=b_c_col[:, :], scale=1.0)
        # attn[b, h]
        ps_ab = sc_psum.tile([B, H], F32, tag="sc")
        nc.tensor.matmul(ps_ab[:, :], lhsT=attnT[:, :], rhs=identity[:, :],
                         start=True, stop=True)
        attn = small.tile([B, H], F32, tag="attn")
        nc.vector.tensor_copy(out=attn[:, :], in_=ps_ab[:, :])
        if debug:
            nc.sync.dma_start(out=dbg_sums[:, :], in_=sums_row[:, :])
            nc.sync.dma_start(out=dbg_ctx[:, :], in_=ctxT[:, :])
            nc.sync.dma_start(out=dbg_attn[:, :], in_=attn[:, :])
        # LayerNorm over h (free dim), keras eps inside sqrt
        sum1 = small.tile([B, 1], F32, tag="sum1")
        nc.vector.tensor_reduce(sum1[:, :], attn[:, :], axis=AX_X, op=ADD)
        mean = small.tile([B, 1], F32, tag="mean")
        nc.vector.tensor_scalar_mul(mean[:, :], sum1[:, :], 1.0 / H)
        xc = small.tile([B, H], F32, tag="xc")
        nc.vector.tensor_scalar(out=xc[:, :], in0=attn[:, :],
                                scalar1=mean[:, :], scalar2=None, op0=SUB)
        sq = small.tile([B, H], F32, tag="sq")
        nc.vector.tensor_tensor(out=sq[:, :], in0=xc[:, :], in1=xc[:, :],
                                op=MULT)
        s2 = small.tile([B, 1], F32, tag="s2")
        nc.vector.tensor_reduce(s2[:, :], sq[:, :], axis=AX_X, op=ADD)
        var = small.tile([B, 1], F32, tag="var")
        nc.vector.tensor_scalar_mul(var[:, :], s2[:, :], 1.0 / H)
        std = small.tile([B, 1], F32, tag="std")
        nc.scalar.activation(std[:, :], var[:, :], Sqrt, bias=eps_col[:, :],
                             scale=1.0)
        istd = small.tile([B, 1], F32, tag="istd")
        nc.vector.reciprocal(out=istd[:, :], in_=std[:, :])
        xn = small.tile([B, H], F32, tag="xn")
        nc.vector.tensor_scalar(out=xn[:, :], in0=xc[:, :],
                                scalar1=istd[:, :], scalar2=None, op0=MULT)
        y1 = small.tile([B, H], F32, tag="y1")
        nc.vector.tensor_tensor(out=y1[:, :], in0=xn[:, :], in1=gamma_b[:, :],
                                op=MULT)
        out_t = small.tile([B, H], F32, tag="out_t")
        nc.vector.tensor_tensor(out=out_t[:, :], in0=y1[:, :], in1=beta_b[:, :],
                                op=ADD)
        nc.sync.dma_start(out=out_e[:, :], in_=out_t[:, :])

    _normalize_waits(nc)
    return nc


def _normalize_waits(nc):
    """This walrus build rejects instructions carrying more sync waits than
    their ISA struct allows (and DMA-transpose / Drain structs allow none).
    Move excess waits onto single-wait nops immediately before the
    instruction on the same engine — engine streams are in-order, so this is
    semantically identical."""
    ZERO_WAIT = (mybir.InstDmaTransposeAnt, mybir.InstDrain)
    for blk in nc.main_func.blocks:
        insts = blk.instructions
        idx = 0
        while idx < len(insts):
            inst = insts[idx]
            si = inst.sync_info
            if si is not None:
                if isinstance(inst, ZERO_WAIT):
                    keep = 0
                elif isinstance(inst, mybir.InstEventSemaphore):
                    keep = 2
                else:
                    keep = 1
                waits = list(si.on_wait)
                if len(waits) > keep:
                    for w in waits[keep:]:
                        nop = mybir.InstNoOp(
                            name=nc.get_next_instruction_name(), ins=[], outs=[])
                        nop.engine = inst.engine
                        nop.sync_info = mybir.SyncInfo(on_wait=[w],
                                                       on_update=[])
                        nc.register_instruction(nop)
                        insts.insert(idx, nop)
                        idx += 1
                    si.on_wait = waits[:keep]
            idx += 1


_NC_CACHE = None


def _get_nc():
    global _NC_CACHE
    if _NC_CACHE is None:
        _NC_CACHE = _build()
    return _NC_CACHE


def _make_in_maps(h_t, h_s, W_a, U_a, V_a, W_c, b_c, gamma, beta):
    in_maps = []
    for i in range(NCORES):
        sl = slice(i * B, (i + 1) * B)
        in_maps.append({
            "h_t": np.ascontiguousarray(h_t[sl], dtype=np.float32),
            "h_s": np.ascontiguousarray(h_s[sl], dtype=np.float32),
            "W_a": np.ascontiguousarray(W_a, dtype=np.float32),
            "U_a": np.ascontiguousarray(U_a, dtype=np.float32),
            "V_a": np.ascontiguousarray(V_a, dtype=np.float32),
            "W_c": np.ascontiguousarray(W_c, dtype=np.float32),
            "b_c": np.ascontiguousarray(b_c, dtype=np.float32),
            "gamma": np.ascontiguousarray(gamma, dtype=np.float32),
            "beta": np.ascontiguousarray(beta, dtype=np.float32),
        })
    return in_maps


def run_spmd(trace=False, **inputs):
    """Runs the kernel; returns (full_output, BassKernelResults)."""
    nc = _get_nc()
    in_maps = _make_in_maps(**inputs)
    res = run_bass_kernel_spmd(nc, in_maps, core_ids=list(range(NCORES)),
                               trace=trace)
    out = np.concatenate([res.results[i]["out"] for i in range(NCORES)], axis=0)
    return out.astype(np.float32), res


def kernel(**inputs) -> np.ndarray:
    out, _ = run_spmd(trace=False, **inputs)
    return out


# revision 27
# speedup vs baseline: 1.2495x; 1.2495x over previous
"""Bahdanau additive attention + LayerNorm, distributed over 8 TRN2 NeuronCores.

Data parallel over the batch dim: each core handles 128 of the 1024 batch rows.
Per core (B=128, S=1024, D=H=128):
  - h_s is streamed from HBM once, cast f32->bf16 in-flight by the SWDGE DMA
    (natural [s, d] layout), and a transposed [d, s] copy is produced by the
    DMA xbar transpose so both the U_a projection (contracts d) and the
    context reduction (contracts s) can run on TensorE without any engine
    transpose pass.
  - scores are computed per 128-wide s-chunk with the tanh energy tile as the
    matmul stationary operand, so they land across partitions; softmax runs
    unnormalized (exp then a deferred divide in the epilogue).
"""

import os
import numpy as np
from contextlib import ExitStack

import concourse.bass as bass
import concourse.mybir as mybir
from concourse.bass_utils import run_bass_kernel_spmd
from concourse.tile import TileContext
from concourse.vector_clock import ScopedClock, VectorClock
from concourse.masks import make_identity

# ---------------------------------------------------------------------------
# Workaround for walrus "Too many sync wait commands" on the TileContext final
# Drain: put the end-of-kernel semaphore waits on individual nops (engine
# instructions execute in order, so a bare drain afterwards is equivalent).
# ---------------------------------------------------------------------------


def _patched_drain_and_barrier(self, tick_clock, wait_clock):
    gc = tick_clock.global_clock
    for i, t in enumerate(list(gc)):
        if t > 0:
            pc = VectorClock()
            for _ in range(t):
                pc.advance(i)
            nop_i = self.nc.sync.nop(hint=f"drainwait{i}", nofuse=True)
            wait_clock.add_sem_waits(nop_i.ins, ScopedClock({None: pc}))
    self.nc.sync.drain()
    self.nc.all_engine_barrier()
    assert self.sems is not None
    popped = self.nc._tile_sem_poison_stack.pop()
    assert popped is self._sem_poison
    self.nc.clear_and_free_semaphores(list(self.sems.allocated().values()))
    self.nc.all_engine_barrier()


TileContext._drain_and_barrier = _patched_drain_and_barrier

# (walrus's ldw-opt / FWL is incompatible with this bass's standalone
# InstLdweights lowering — tested, codegen rejects it.)

# ---------------------------------------------------------------------------

NCORES = 8
B = 128          # batch rows per core (1024 / 8)
S = 1024
D = 128
H = 128
EPS = 1e-3

F32 = mybir.dt.float32
BF16 = mybir.dt.bfloat16

GRP = 16         # batch rows per softmax group (one psum scores bank)
NCH = S // 128   # 8 s-chunks

Tanh = mybir.ActivationFunctionType.Tanh
Exp = mybir.ActivationFunctionType.Exp
Sqrt = mybir.ActivationFunctionType.Sqrt
ADD = mybir.AluOpType.add
SUB = mybir.AluOpType.subtract
MULT = mybir.AluOpType.mult
DIV = mybir.AluOpType.divide
AX_X = mybir.AxisListType.X


def _build(debug=False):
    nc = bass.Bass()
    h_t_e = nc.declare_dram_parameter("h_t", [B, D], F32, isOutput=False)
    h_s_e = nc.declare_dram_parameter("h_s", [B, S, D], F32, isOutput=False)
    W_a_e = nc.declare_dram_parameter("W_a", [D, H], F32, isOutput=False)
    U_a_e = nc.declare_dram_parameter("U_a", [D, H], F32, isOutput=False)
    V_a_e = nc.declare_dram_parameter("V_a", [H, 1], F32, isOutput=False)
    W_c_e = nc.declare_dram_parameter("W_c", [2 * D, H], F32, isOutput=False)
    b_c_e = nc.declare_dram_parameter("b_c", [H], F32, isOutput=False)
    gamma_e = nc.declare_dram_parameter("gamma", [H], F32, isOutput=False)
    beta_e = nc.declare_dram_parameter("beta", [H], F32, isOutput=False)
    out_e = nc.declare_dram_parameter("out", [B, H], F32, isOutput=True)
    if debug:
        dbg_hsT = nc.declare_dram_parameter("dbg_hsT", [D, S], F32, isOutput=True)
        dbg_tanh = nc.declare_dram_parameter("dbg_tanh", [H, S], F32, isOutput=True)
        dbg_exp = nc.declare_dram_parameter("dbg_exp", [128, 128], F32, isOutput=True)
        dbg_sums = nc.declare_dram_parameter("dbg_sums", [1, B], F32, isOutput=True)
        dbg_ctx = nc.declare_dram_parameter("dbg_ctx", [B, D], F32, isOutput=True)
        dbg_attn = nc.declare_dram_parameter("dbg_attn", [B, H], F32, isOutput=True)

    with TileContext(nc) as tc, ExitStack() as ctx:
        consts = ctx.enter_context(tc.tile_pool(name="consts", bufs=1))
        nat_pool = ctx.enter_context(tc.tile_pool(name="nat", bufs=24))
        hsT_pool = ctx.enter_context(tc.tile_pool(name="hsT", bufs=4))
        tanh_pool = ctx.enter_context(tc.tile_pool(name="tanh", bufs=4))
        exp_pool = ctx.enter_context(tc.tile_pool(name="expT", bufs=2))
        small = ctx.enter_context(tc.tile_pool(name="small", bufs=4))
        pe_psum = ctx.enter_context(tc.tile_pool(name="pe_psum", bufs=2, space="PSUM"))
        sc_psum = ctx.enter_context(tc.tile_pool(name="sc_psum", bufs=2, space="PSUM"))
        tr_psum = ctx.enter_context(tc.tile_pool(name="tr_psum", bufs=2, space="PSUM"))

        # ----- constants / preamble -----
        identity = consts.tile([128, 128], F32, tag="identity")
        make_identity(nc, identity)
        ident_bf = consts.tile([128, 128], BF16, tag="ident_bf")
        make_identity(nc, ident_bf)
        ones_col = consts.tile([128, 1], F32, tag="ones_col")
        nc.vector.memset(ones_col, 1.0)
        ones_row = consts.tile([1, 128], F32, tag="ones_row")
        nc.vector.memset(ones_row, 1.0)
        eps_col = consts.tile([128, 1], F32, tag="eps_col")
        nc.vector.memset(eps_col, EPS)

        U_bf = consts.tile([D, H], BF16, tag="U_bf")
        nc.gpsimd.dma_start(out=U_bf[:, :], in_=U_a_e[:, :])
        V_bf = consts.tile([H, 1], BF16, tag="V_bf")
        nc.gpsimd.dma_start(out=V_bf[:, :], in_=V_a_e[:, :])
        W_a_s = consts.tile([D, H], F32, tag="W_a_s")
        nc.sync.dma_start(out=W_a_s[:, :], in_=W_a_e[:, :])
        Wc_top = consts.tile([D, H], F32, tag="Wc_top")
        nc.sync.dma_start(out=Wc_top[:, :], in_=W_c_e[0:D, :])
        Wc_bot = consts.tile([D, H], F32, tag="Wc_bot")
        nc.sync.dma_start(out=Wc_bot[:, :], in_=W_c_e[D : 2 * D, :])
        b_c_col = consts.tile([H, 1], F32, tag="b_c_col")
        nc.sync.dma_start(out=b_c_col[:, :], in_=b_c_e[:])
        gamma_row = consts.tile([1, H], F32, tag="gamma_row")
        nc.sync.dma_start(out=gamma_row[:, :], in_=gamma_e[:])
        beta_row = consts.tile([1, H], F32, tag="beta_row")
        nc.sync.dma_start(out=beta_row[:, :], in_=beta_e[:])
        h_t_s = consts.tile([B, D], F32, tag="h_t_s")
        nc.sync.dma_start(out=h_t_s[:, :], in_=h_t_e[:, :])

        # broadcast gamma/beta across partitions: ones[128,1] (x) row[1,128]
        ps_g = sc_psum.tile([128, H], F32, tag="sc")
        nc.tensor.matmul(ps_g[:, :], lhsT=ones_row[:, :], rhs=gamma_row[:, :],
                         start=True, stop=True)
        gamma_b = consts.tile([128, H], F32, tag="gamma_b")
        nc.vector.tensor_copy(out=gamma_b[:, :], in_=ps_g[:, :])
        ps_b = sc_psum.tile([128, H], F32, tag="sc")
        nc.tensor.matmul(ps_b[:, :], lhsT=ones_row[:, :], rhs=beta_row[:, :],
                         start=True, stop=True)
        beta_b = consts.tile([128, H], F32, tag="beta_b")
        nc.vector.tensor_copy(out=beta_b[:, :], in_=ps_b[:, :])

        # htT[d, b] = h_t^T ; ht_projT[h, b] = W_a^T @ htT
        ps_t = sc_psum.tile([D, B], F32, tag="sc")
        nc.tensor.matmul(ps_t[:, :], lhsT=h_t_s[:, :], rhs=identity[:, :],
                         start=True, stop=True)
        htT = consts.tile([D, B], F32, tag="htT")
        nc.vector.tensor_copy(out=htT[:, :], in_=ps_t[:, :])
        ps_p = sc_psum.tile([H, B], F32, tag="sc")
        nc.tensor.matmul(ps_p[:, :], lhsT=W_a_s[:, :], rhs=htT[:, :],
                         start=True, stop=True)
        ht_projT = consts.tile([H, B], F32, tag="ht_projT")
        nc.vector.tensor_copy(out=ht_projT[:, :], in_=ps_p[:, :])

        # persistent accumulators
        ctxT_raw = consts.tile([D, B], F32, tag="ctxT_raw")
        sums_row = consts.tile([1, B], F32, tag="sums_row")

        # ----- main loop over softmax groups of GRP batch rows -----
        # Group psum bank layout (one [128, 512] tile per group, ALL matmuls
        # single-shot so the per-bank has_written bit clearing is harmless):
        #   cols [0, 128)    scoresT: col bl*8+c = scores of row bl, s-chunk c
        #   cols [128, 144)  per-row exp sums (row 0 only)
        #   cols [144, 272)  ctx partials: col 144+bl*8+c = partial ctx^T
        n_groups = B // GRP
        for g in range(n_groups):
            ps_grp = sc_psum.tile([128, 512], F32, tag="sc")
            expT_g = exp_pool.tile([128, GRP * 8], BF16, tag="expT")
            nat_tiles = []
            for bl in range(GRP):
                b = g * GRP + bl
                # 1. HBM load with f32->bf16 cast. Partition p holds the 8
                # consecutive rows s = 8p..8p+7 (4KB contiguous source per
                # partition -> full-size DMA descriptors); "chunk" c is the
                # strided subset {s : s % 8 == c}. Softmax and the context
                # sum are s-order-agnostic, and scores/exp/ctx all use the
                # same chunk labeling, so the permutation is invisible.
                nat_b = nat_pool.tile([128, NCH, D], BF16, tag="nat")
                nc.gpsimd.dma_start(
                    out=nat_b[:, :, :],
                    in_=h_s_e[b].rearrange("(p r) d -> p r d", r=NCH),
                )
                nat_tiles.append(nat_b)
                # 2. transpose via TensorE transpose-mode (bf16 psum),
                #    evacuating psum->SBUF on DVE
                hsT = hsT_pool.tile([D, S], BF16, tag="hsT")
                for half in range(2):
                    ps_tr = tr_psum.tile([D, 512], BF16, tag="tr")
                    for cc in range(4):
                        c = half * 4 + cc
                        nc.tensor.transpose(
                            ps_tr[:, cc * 128 : (cc + 1) * 128],
                            nat_b[:, c, :], ident_bf[:, :],
                        )
                    dst = hsT[:, half * 512 : (half + 1) * 512]
                    nc.vector.tensor_copy(out=dst, in_=ps_tr[:, :])
                # 3. energy pre-activation: U_a^T @ h_s^T -> [h, s]
                ps_e = pe_psum.tile([H, S], F32, tag="pe")
                nc.tensor.matmul(ps_e[:, 0:512], lhsT=U_bf[:, :],
                                 rhs=hsT[:, 0:512], start=True, stop=True)
                nc.tensor.matmul(ps_e[:, 512:1024], lhsT=U_bf[:, :],
                                 rhs=hsT[:, 512:1024], start=True, stop=True)
                # 4. tanh(e + ht_proj[b]) with per-partition bias
                tanh_e = tanh_pool.tile([H, S], BF16, tag="tanh")
                nc.scalar.activation(tanh_e[:, :], ps_e[:, :], Tanh,
                                     bias=ht_projT[:, b : b + 1], scale=1.0)
                if debug and b == 0:
                    nc.gpsimd.dma_start(out=dbg_hsT[:, :], in_=hsT[:, :])
                    nc.gpsimd.dma_start(out=dbg_tanh[:, :], in_=tanh_e[:, :])
                # 5. scores: V_a^T tanh_e per chunk; stationary = tanh chunk
                for c in range(NCH):
                    k = bl * 8 + c
                    nc.tensor.matmul(
                        ps_grp[:, k : k + 1],
                        lhsT=tanh_e[:, c * 128 : (c + 1) * 128],
                        rhs=V_bf[:, :],
                        start=True, stop=True,
                    )
            # group tail: exp, per-row sums, context partials
            nc.scalar.activation(expT_g[:, :], ps_grp[:, 0 : GRP * 8], Exp)
            sumP = small.tile([128, GRP], F32, tag="sumP")
            nc.vector.tensor_reduce(
                sumP[:, :],
                expT_g.rearrange("p (b c) -> p b c", c=8),
                axis=AX_X, op=ADD,
            )
            nc.tensor.matmul(ps_grp[0:1, 128 : 128 + GRP], lhsT=ones_col[:, :],
                             rhs=sumP[:, :], start=True, stop=True)
            sums_cp = nc.vector.tensor_copy(
                out=sums_row[:, g * GRP : (g + 1) * GRP],
                in_=ps_grp[0:1, 128 : 128 + GRP])
            if debug and g == 0:
                nc.gpsimd.dma_start(out=dbg_exp[:, :], in_=expT_g[:, :])
            # ctx^T partials: one column per (row, chunk), no accumulation.
            # TensorE writes here must not overlap the DVE read of the sums
            # region in the same psum bank (PSUM collisions are fatal).
            first_ctx = True
            for bl in range(GRP):
                nat_b = nat_tiles[bl]
                for c in range(NCH):
                    k = 144 + bl * 8 + c
                    mm = nc.tensor.matmul(
                        ps_grp[:, k : k + 1],
                        lhsT=nat_b[:, c, :],
                        rhs=expT_g[:, bl * 8 + c : bl * 8 + c + 1],
                        start=True, stop=True,
                    )
                    if first_ctx:
                        bass._add_dep_helper(
                            mm.ins, sums_cp.ins, sync=True,
                            reason="ctx psum writes wait for sums bank read")
                        first_ctx = False
            # reduce the 8 partials per row -> ctx^T[:, g*16:(g+1)*16]
            nc.vector.tensor_reduce(
                ctxT_raw[:, g * GRP : (g + 1) * GRP],
                ps_grp[:, 144 : 144 + GRP * 8].rearrange("p (b c) -> p b c", c=8),
                axis=AX_X, op=ADD,
            )

        # ----- epilogue -----
        # normalize ctx^T by the per-row softmax sums (broadcast along d)
        inv_row = small.tile([1, B], F32, tag="inv_row")
        nc.vector.reciprocal(out=inv_row[:, :], in_=sums_row[:, :])
        ps_ib = sc_psum.tile([128, B], F32, tag="sc")
        nc.tensor.matmul(ps_ib[:, :], lhsT=ones_row[:, :], rhs=inv_row[:, :],
                         start=True, stop=True)
        ctxT = small.tile([D, B], F32, tag="ctxT")
        nc.vector.tensor_tensor(out=ctxT[:, :], in0=ctxT_raw[:, :],
                                in1=ps_ib[:, :], op=MULT)
        # attnT[h, b] = tanh(Wc_top^T ctxT + Wc_bot^T htT + b_c)
        ps_at = sc_psum.tile([H, B], F32, tag="sc")
        nc.tensor.matmul(ps_at[:, :], lhsT=Wc_top[:, :], rhs=ctxT[:, :],
                         start=True, stop=False)
        nc.tensor.matmul(ps_at[:, :], lhsT=Wc_bot[:, :], rhs=htT[:, :],
                         start=False, stop=True)
        attnT = small.tile([H, B], F32, tag="attnT")
        nc.scalar.activation(attnT[:, :], ps_at[:, :], Tanh,
                             bias=b_c_col[:, :], scale=1.0)
        # attn[b, h]
        ps_ab = sc_psum.tile([B, H], F32, tag="sc")
        nc.tensor.matmul(ps_ab[:, :], lhsT=attnT[:, :], rhs=identity[:, :],
                         start=True, stop=True)
        attn = small.tile([B, H], F32, tag="attn")
        nc.vector.tensor_copy(out=attn[:, :], in_=ps_ab[:, :])
        if debug:
            nc.sync.dma_start(out=dbg_sums[:, :], in_=sums_row[:, :])
            nc.sync.dma_start(out=dbg_ctx[:, :], in_=ctxT[:, :])
            nc.sync.dma_start(out=dbg_attn[:, :], in_=attn[:, :])
        # LayerNorm over h (free dim), keras eps inside sqrt
        sum1 = small.tile([B, 1], F32, tag="sum1")
        nc.vector.tensor_reduce(sum1[:, :], attn[:, :], axis=AX_X, op=ADD)
        mean = small.tile([B, 1], F32, tag="mean")
        nc.vector.tensor_scalar_mul(mean[:, :], sum1[:, :], 1.0 / H)
        xc = small.tile([B, H], F32, tag="xc")
        nc.vector.tensor_scalar(out=xc[:, :], in0=attn[:, :],
                                scalar1=mean[:, :], scalar2=None, op0=SUB)
        sq = small.tile([B, H], F32, tag="sq")
        nc.vector.tensor_tensor(out=sq[:, :], in0=xc[:, :], in1=xc[:, :],
                                op=MULT)
        s2 = small.tile([B, 1], F32, tag="s2")
        nc.vector.tensor_reduce(s2[:, :], sq[:, :], axis=AX_X, op=ADD)
        var = small.tile([B, 1], F32, tag="var")
        nc.vector.tensor_scalar_mul(var[:, :], s2[:, :], 1.0 / H)
        std = small.tile([B, 1], F32, tag="std")
        nc.scalar.activation(std[:, :], var[:, :], Sqrt, bias=eps_col[:, :],
                             scale=1.0)
        istd = small.tile([B, 1], F32, tag="istd")
        nc.vector.reciprocal(out=istd[:, :], in_=std[:, :])
        xn = small.tile([B, H], F32, tag="xn")
        nc.vector.tensor_scalar(out=xn[:, :], in0=xc[:, :],
                                scalar1=istd[:, :], scalar2=None, op0=MULT)
        y1 = small.tile([B, H], F32, tag="y1")
        nc.vector.tensor_tensor(out=y1[:, :], in0=xn[:, :], in1=gamma_b[:, :],
                                op=MULT)
        out_t = small.tile([B, H], F32, tag="out_t")
        nc.vector.tensor_tensor(out=out_t[:, :], in0=y1[:, :], in1=beta_b[:, :],
                                op=ADD)
        nc.sync.dma_start(out=out_e[:, :], in_=out_t[:, :])

    _normalize_waits(nc)
    return nc


def _normalize_waits(nc):
    """This walrus build rejects instructions carrying more sync waits than
    their ISA struct allows (and DMA-transpose / Drain structs allow none).
    Move excess waits onto single-wait nops immediately before the
    instruction on the same engine — engine streams are in-order, so this is
    semantically identical."""
    ZERO_WAIT = (mybir.InstDmaTransposeAnt, mybir.InstDrain)
    for blk in nc.main_func.blocks:
        insts = blk.instructions
        idx = 0
        while idx < len(insts):
            inst = insts[idx]
            si = inst.sync_info
            if si is not None:
                if isinstance(inst, ZERO_WAIT):
                    keep = 0
                elif isinstance(inst, mybir.InstEventSemaphore):
                    keep = 2
                else:
                    keep = 1
                waits = list(si.on_wait)
                if len(waits) > keep:
                    for w in waits[keep:]:
                        nop = mybir.InstNoOp(
                            name=nc.get_next_instruction_name(), ins=[], outs=[])
                        nop.engine = inst.engine
                        nop.sync_info = mybir.SyncInfo(on_wait=[w],
                                                       on_update=[])
                        nc.register_instruction(nop)
                        insts.insert(idx, nop)
                        idx += 1
                    si.on_wait = waits[:keep]
            idx += 1


_NC_CACHE = None


def _get_nc():
    global _NC_CACHE
    if _NC_CACHE is None:
        _NC_CACHE = _build()
    return _NC_CACHE


def _make_in_maps(h_t, h_s, W_a, U_a, V_a, W_c, b_c, gamma, beta):
    in_maps = []
    for i in range(NCORES):
        sl = slice(i * B, (i + 1) * B)
        in_maps.append({
            "h_t": np.ascontiguousarray(h_t[sl], dtype=np.float32),
            "h_s": np.ascontiguousarray(h_s[sl], dtype=np.float32),
            "W_a": np.ascontiguousarray(W_a, dtype=np.float32),
            "U_a": np.ascontiguousarray(U_a, dtype=np.float32),
            "V_a": np.ascontiguousarray(V_a, dtype=np.float32),
            "W_c": np.ascontiguousarray(W_c, dtype=np.float32),
            "b_c": np.ascontiguousarray(b_c, dtype=np.float32),
            "gamma": np.ascontiguousarray(gamma, dtype=np.float32),
            "beta": np.ascontiguousarray(beta, dtype=np.float32),
        })
    return in_maps


def run_spmd(trace=False, **inputs):
    """Runs the kernel; returns (full_output, BassKernelResults)."""
    nc = _get_nc()
    in_maps = _make_in_maps(**inputs)
    res = run_bass_kernel_spmd(nc, in_maps, core_ids=list(range(NCORES)),
                               trace=trace)
    out = np.concatenate([res.results[i]["out"] for i in range(NCORES)], axis=0)
    return out.astype(np.float32), res


def kernel(**inputs) -> np.ndarray:
    out, _ = run_spmd(trace=False, **inputs)
    return out
